# Initial kernel scaffold
#
"""Bass/Tile kernel for masked multi-head attention on 8 trn2 NeuronCores.

Problem (hardcoded shapes): B=4, S=2048, DM=1024, H=16, D=64.
  q = Q_seq @ WQ, k = K_seq @ WK, v = V_seq @ WV  (per-head split, D=64)
  A = softmax(q k^T / 8  masked to keys < V_len[b])
  O = (A v) masked to queries < Q_len[b]

Sharding: core c owns head pair hp=c (heads 2c, 2c+1) of EVERY batch.
All cores run an identical (SPMD) program; per-core data = W column slices.
This balances attention, projection and DMA work exactly 8 ways.

Device layout tricks:
  - Host pre-transposes Q/K/V to DM-major [1024, S] so every projection
    matmul contracts over partitions with clean DMAs.
  - scores are computed TRANSPOSED (keys on partitions) so the AV matmul
    can consume them directly; softmax denominators come from a ones-vector
    matmul; exp never needs a max-subtraction (scores are O(1) here, and
    masked keys are excluded exactly via zeroed V rows + masked ones).
  - 1/sqrt(D) is folded into WQ on the host. bf16 operands, fp32 PSUM.
  - Output is returned as unnormalized O^T + denominators; the host
    divides, transposes back, applies the query mask and assembles.
"""

import math
import os

import ml_dtypes
import numpy as np

B, S, DM, H, D = 4, 2048, 1024, 16, 64
P = 128
NCORES = 8
QSPAN = 512  # query span per attention cell (free dim of score matmuls)

LAST_EXEC_NS = None
LAST_RESULTS = None

_PROGRAM_CACHE = {}


def _ceil(a, b):
    return -(-a // b)


def _split_excess_waits(nc, mybir):
    """Move semaphore waits beyond each instruction's encoding limit onto
    preceding same-engine NoOps.  This walrus build rejects DMACopy (and any
    Pool-engine op) carrying more than one sync wait ("Too many sync wait
    commands"), but an engine-level NoOp can hold the wait instead — the
    engine stalls on the NoOp, then issues the real instruction."""
    uid = 0
    for fn in nc.m.functions:
        for blk in fn.blocks:
            insts = blk.instructions
            out = []
            changed = False
            for inst in insts:
                si = inst.sync_info
                waits = list(si.on_wait) if si is not None and si.on_wait else []
                limit = 1
                if len(waits) > limit:
                    for w in waits[:-limit] if limit else waits:
                        nop = mybir.InstNoOp(name=f"wsplit-{uid}", ins=[],
                                             outs=[])
                        uid += 1
                        nop.engine = inst.engine
                        nop.sync_info = mybir.SyncInfo(on_wait=[w],
                                                       on_update=[])
                        out.append(nop)
                    si.on_wait = waits[-limit:] if limit else []
                    changed = True
                out.append(inst)
            if changed:
                blk.instructions = out


def _build_program(qlen, vlen):
    """Build the SPMD Bass program for the given per-batch lengths."""
    import concourse.bass as bass
    import concourse.mybir as mybir
    import concourse.tile as tile

    bf16 = mybir.dt.bfloat16
    f32 = mybir.dt.float32
    AF = mybir.ActivationFunctionType

    spans = [(_ceil(qlen[b], QSPAN) if vlen[b] > 0 else 0) for b in range(B)]
    kspans = [_ceil(vlen[b], 512) for b in range(B)]
    ktn = [_ceil(vlen[b], P) for b in range(B)]
    active = [b for b in range(B) if spans[b] > 0 and ktn[b] > 0]

    nc = bass.Bass(
        "TRN2",
        target_bir_lowering=False,
        debug=False,
        enable_asserts=False,
        num_devices=NCORES,
    )

    # ---- DRAM tensors ----
    qt_d, kt_d, vt_d, ot_d, den_d = {}, {}, {}, {}, {}
    for b in active:
        qt_d[b] = nc.dram_tensor(f"qt{b}", [DM, spans[b] * QSPAN], bf16,
                                 kind="ExternalInput").ap()
        kt_d[b] = nc.dram_tensor(f"kt{b}", [DM, kspans[b] * 512], bf16,
                                 kind="ExternalInput").ap()
        vt_d[b] = nc.dram_tensor(f"vt{b}", [DM, ktn[b] * P], bf16,
                                 kind="ExternalInput").ap()
        ot_d[b] = nc.dram_tensor(f"ot{b}", [spans[b], P, QSPAN], f32,
                                 kind="ExternalOutput").ap()
        den_d[b] = nc.dram_tensor(f"den{b}", [spans[b], 33, QSPAN], f32,
                                  kind="ExternalOutput").ap()
    wq_d = nc.dram_tensor("wq", [DM, P], bf16, kind="ExternalInput").ap()
    wk_d = nc.dram_tensor("wk", [DM, P], bf16, kind="ExternalInput").ap()
    wv_d = nc.dram_tensor("wv", [DM, P], bf16, kind="ExternalInput").ap()

    with tile.TileContext(nc) as tc:
        with (
            tc.tile_pool(name="wpool", bufs=1) as wpool,
            tc.tile_pool(name="proj", bufs=1) as projpool,
            tc.tile_pool(name="stage", bufs=1) as stage,
            tc.tile_pool(name="sbig", bufs=3) as sbig,
            tc.tile_pool(name="outp", bufs=2) as outp,
            tc.tile_pool(name="pproj", bufs=2, space="PSUM") as pproj,
            tc.tile_pool(name="psc", bufs=2, space="PSUM") as psc,
            tc.tile_pool(name="ppo", bufs=1, space="PSUM") as ppo,
            tc.tile_pool(name="ppd", bufs=1, space="PSUM") as ppd,
        ):
            # Weights for this core's head pair, DM on partitions: [128, 8, 128]
            w_sb = {}
            for name, ap in (("wq", wq_d), ("wk", wk_d), ("wv", wv_d)):
                t = wpool.tile([P, DM // P, P], bf16, tag=f"w_{name}")
                nc.sync.dma_start(t, ap.rearrange("(c p) m -> p c m", p=P))
                w_sb[name] = t
            ones_sb = wpool.tile([P, 1], bf16, tag="ones")
            nc.vector.memset(ones_sb, 1.0)

            for b in active:
                nsp, nksp, nkt = spans[b], kspans[b], ktn[b]
                boundary = nkt - 1 if vlen[b] % P != 0 else -1
                kones_sb = wpool.tile([P, 1], bf16, tag="kones")
                if boundary >= 0:
                    r = vlen[b] - (nkt - 1) * P
                    nc.vector.memset(kones_sb, 0.0)
                    nc.vector.memset(kones_sb[0:r], 1.0)

                qT = projpool.tile([P, nsp * QSPAN], bf16, tag="qT")
                kT = projpool.tile([P, nksp * 512], bf16, tag="kT")
                vn = projpool.tile([P, nkt * P], bf16, tag="vn")

                # Stage full transposed activations with one DMA each.
                qst = stage.tile([P, DM // P, nsp * QSPAN], bf16, tag="qst")
                nc.sync.dma_start(qst, qt_d[b].rearrange("(c p) n -> p c n",
                                                         p=P))
                kst = stage.tile([P, DM // P, nksp * 512], bf16, tag="kst")
                nc.sync.dma_start(kst, kt_d[b].rearrange("(c p) n -> p c n",
                                                         p=P))
                vst = stage.tile([P, DM // P, nkt * P], bf16, tag="vst")
                nc.sync.dma_start(vst, vt_d[b].rearrange("(c p) n -> p c n",
                                                         p=P))

                # q projection: qT[:, span] = (WQ_hp^T @ Q^T)[128, 512]
                for sp in range(nsp):
                    ps = pproj.tile([P, 512], f32, tag="proj")
                    for ch in range(DM // P):
                        nc.tensor.matmul(ps[:, :QSPAN], lhsT=w_sb["wq"][:, ch],
                                         rhs=qst[:, ch,
                                                 sp * QSPAN:(sp + 1) * QSPAN],
                                         start=(ch == 0),
                                         stop=(ch == DM // P - 1))
                    nc.vector.tensor_copy(qT[:, sp * QSPAN:(sp + 1) * QSPAN],
                                          ps[:, :QSPAN])
                # k projection
                for sp in range(nksp):
                    ps = pproj.tile([P, 512], f32, tag="proj")
                    for ch in range(DM // P):
                        nc.tensor.matmul(ps, lhsT=w_sb["wk"][:, ch],
                                         rhs=kst[:, ch,
                                                 sp * 512:(sp + 1) * 512],
                                         start=(ch == 0),
                                         stop=(ch == DM // P - 1))
                    nc.vector.tensor_copy(kT[:, sp * 512:(sp + 1) * 512], ps)
                # v projection: v natural [128 kpos, 128 hp-dims] per ktile
                for kt in range(nkt):
                    ps = pproj.tile([P, 512], f32, tag="proj")
                    for ch in range(DM // P):
                        nc.tensor.matmul(ps[:, :P],
                                         lhsT=vst[:, ch, kt * P:(kt + 1) * P],
                                         rhs=w_sb["wv"][:, ch],
                                         start=(ch == 0),
                                         stop=(ch == DM // P - 1))
                    nc.vector.tensor_copy(vn[:, kt * P:(kt + 1) * P],
                                          ps[:, :P])

                # attention
                for sp in range(nsp):
                    qs = slice(sp * QSPAN, (sp + 1) * QSPAN)
                    po = ppo.tile([P, QSPAN], f32, tag="po")
                    pd = ppd.tile([33, QSPAN], f32, tag="pd")
                    for kt in range(nkt):
                        ks = slice(kt * P, (kt + 1) * P)
                        psc_t = psc.tile([P, 2 * QSPAN], f32, tag="sc")
                        # scoresT = kT_slice^T @ qT : [128 kpos, 512 q], 2 heads
                        nc.tensor.matmul(psc_t[:, 0:QSPAN],
                                         lhsT=kT[0:64, ks], rhs=qT[0:64, qs],
                                         start=True, stop=True,
                                         tile_position=(0, 0))
                        nc.tensor.matmul(psc_t[:, QSPAN:2 * QSPAN],
                                         lhsT=kT[64:P, ks], rhs=qT[64:P, qs],
                                         start=True, stop=True,
                                         tile_position=(64, 0))
                        ex = sbig.tile([P, 2 * QSPAN], bf16, tag="exp")
                        nc.scalar.activation(ex, psc_t, AF.Exp)
                        first, last = (kt == 0), (kt == nkt - 1)
                        ov = kones_sb if kt == boundary else ones_sb
                        nc.tensor.matmul(po[0:64, :],
                                         lhsT=vn[:, kt * P:kt * P + 64],
                                         rhs=ex[:, 0:QSPAN],
                                         start=first, stop=last,
                                         tile_position=(0, 0))
                        nc.tensor.matmul(po[64:P, :],
                                         lhsT=vn[:, kt * P + 64:kt * P + P],
                                         rhs=ex[:, QSPAN:2 * QSPAN],
                                         start=first, stop=last,
                                         tile_position=(0, 64))
                        nc.tensor.matmul(pd[0:1, :], lhsT=ov,
                                         rhs=ex[:, 0:QSPAN],
                                         start=first, stop=last,
                                         tile_position=(0, 0))
                        nc.tensor.matmul(pd[32:33, :], lhsT=ov,
                                         rhs=ex[:, QSPAN:2 * QSPAN],
                                         start=first, stop=last,
                                         tile_position=(0, 32))
                    osb = outp.tile([P, QSPAN], f32, tag="osb")
                    nc.vector.tensor_copy(osb, po)
                    nc.sync.dma_start(ot_d[b][sp], osb)
                    dsb = outp.tile([33, QSPAN], f32, tag="dsb")
                    nc.vector.tensor_copy(dsb[0:1], pd[0:1])
                    nc.vector.tensor_copy(dsb[32:33], pd[32:33])
                    nc.sync.dma_start(den_d[b][sp], dsb)

    _split_excess_waits(nc, mybir)
    return nc, spans, kspans, ktn, active


def kernel(Q_seq, K_seq, V_seq, Q_len, V_len, WQ, WK, WV):
    global LAST_EXEC_NS, LAST_RESULTS
    import concourse.bass_utils as bass_utils

    Q_seq = np.ascontiguousarray(np.asarray(Q_seq, dtype=np.float32))
    K_seq = np.ascontiguousarray(np.asarray(K_seq, dtype=np.float32))
    V_seq = np.ascontiguousarray(np.asarray(V_seq, dtype=np.float32))
    WQ = np.asarray(WQ, dtype=np.float32)
    WK = np.asarray(WK, dtype=np.float32)
    WV = np.asarray(WV, dtype=np.float32)
    qlen = [int(x) for x in np.asarray(Q_len).ravel()]
    vlen = [int(x) for x in np.asarray(V_len).ravel()]

    bf = ml_dtypes.bfloat16
    out = np.zeros((B, S, H * D), dtype=np.float32)

    # Degenerate batches (V_len==0): reference softmax of an all-masked row
    # is uniform over all S keys -> O row = mean of v rows.
    host_b = [b for b in range(B)
              if vlen[b] == 0 and qlen[b] > 0]
    for b in host_b:
        v = V_seq[b] @ WV
        out[b, :qlen[b], :] = v.mean(axis=0, keepdims=True)

    key = (tuple(qlen), tuple(vlen))
    if key not in _PROGRAM_CACHE:
        _PROGRAM_CACHE[key] = _build_program(qlen, vlen)
    nc, spans, kspans, ktn, active = _PROGRAM_CACHE[key]

    if active:
        WQs = (WQ / math.sqrt(D)).astype(bf)
        WKs = WK.astype(bf)
        WVs = WV.astype(bf)

        # Shared (core-independent) transposed activations, zero-padded.
        shared = {}
        for b in active:
            qt = np.zeros((DM, spans[b] * QSPAN), dtype=bf)
            qt[:, :qlen[b]] = Q_seq[b, :qlen[b], :].T
            kt = np.zeros((DM, kspans[b] * 512), dtype=bf)
            kt[:, :vlen[b]] = K_seq[b, :vlen[b], :].T
            vt = np.zeros((DM, ktn[b] * P), dtype=bf)
            vt[:, :vlen[b]] = V_seq[b, :vlen[b], :].T
            shared[f"qt{b}"] = qt
            shared[f"kt{b}"] = kt
            shared[f"vt{b}"] = vt

        in_maps = []
        for c in range(NCORES):
            m = dict(shared)
            sl = slice(c * P, (c + 1) * P)
            m["wq"] = np.ascontiguousarray(WQs[:, sl])
            m["wk"] = np.ascontiguousarray(WKs[:, sl])
            m["wv"] = np.ascontiguousarray(WVs[:, sl])
            in_maps.append(m)

        trace = bool(int(os.environ.get("KERNEL_TRACE", "0")))
        res = bass_utils.run_bass_kernel_spmd(
            nc, in_maps, core_ids=list(range(NCORES)), trace=trace)
        LAST_EXEC_NS = res.exec_time_ns
        LAST_RESULTS = res

        for c in range(NCORES):
            r = res.results[c]
            for b in active:
                ot = r[f"ot{b}"]    # [nsp, 128, 512] unnormalized O^T pair
                den = r[f"den{b}"]  # [nsp, 33, 512]; rows 0 and 32 are real
                for sp in range(spans[b]):
                    q0 = sp * QSPAN
                    n = min(qlen[b], q0 + QSPAN) - q0
                    if n <= 0:
                        continue
                    for h in (0, 1):
                        head = 2 * c + h
                        num = ot[sp, h * 64:(h + 1) * 64, :n]
                        d = den[sp, 32 * h, :n]
                        out[b, q0:q0 + n, head * 64:(head + 1) * 64] = \
                            (num / d[None, :]).T
    return out



# revision 4
# speedup vs baseline: 1.0555x; 1.0555x over previous
"""Bass/Tile kernel for masked multi-head attention on 8 trn2 NeuronCores.

Problem (hardcoded shapes): B=4, S=2048, DM=1024, H=16, D=64.
  q = Q_seq @ WQ, k = K_seq @ WK, v = V_seq @ WV  (per-head split, D=64)
  A = softmax(q k^T / 8  masked to keys < V_len[b])
  O = (A v) masked to queries < Q_len[b]

Sharding: core c owns head pair hp=c (heads 2c, 2c+1) of EVERY batch.
All cores run an identical (SPMD) program; per-core data = W column slices.
This balances attention, projection and DMA work exactly 8 ways.

Device layout tricks:
  - Host pre-transposes Q/K/V to DM-major [1024, S] so every projection
    matmul contracts over partitions with clean DMAs.
  - scores are computed TRANSPOSED (keys on partitions) so the AV matmul
    can consume them directly; softmax denominators come from a ones-vector
    matmul; exp never needs a max-subtraction (scores are O(1) here, and
    masked keys are excluded exactly via zeroed V rows + masked ones).
  - 1/sqrt(D) is folded into WQ on the host. bf16 operands, fp32 PSUM.
  - Output is returned as unnormalized O^T + denominators; the host
    divides, transposes back, applies the query mask and assembles.
"""

import math
import os

import ml_dtypes
import numpy as np

B, S, DM, H, D = 4, 2048, 1024, 16, 64
P = 128
NCORES = 8
QSPAN = 512  # query span per attention cell (free dim of score matmuls)

LAST_EXEC_NS = None
LAST_RESULTS = None
LAST_NC = None
LAST_IN_MAPS = None

_PROGRAM_CACHE = {}


def _ceil(a, b):
    return -(-a // b)


def _split_excess_waits(nc, mybir):
    """Move semaphore waits beyond each instruction's encoding limit onto
    preceding same-engine NoOps.  This walrus build rejects DMACopy (and any
    Pool-engine op) carrying more than one sync wait ("Too many sync wait
    commands"), but an engine-level NoOp can hold the wait instead — the
    engine stalls on the NoOp, then issues the real instruction."""
    uid = 0
    for fn in nc.m.functions:
        for blk in fn.blocks:
            insts = blk.instructions
            out = []
            changed = False
            for inst in insts:
                si = inst.sync_info
                waits = list(si.on_wait) if si is not None and si.on_wait else []
                limit = 1
                if len(waits) > limit:
                    for w in waits[:-limit] if limit else waits:
                        nop = mybir.InstNoOp(name=f"wsplit-{uid}", ins=[],
                                             outs=[])
                        uid += 1
                        nop.engine = inst.engine
                        nop.sync_info = mybir.SyncInfo(on_wait=[w],
                                                       on_update=[])
                        out.append(nop)
                    si.on_wait = waits[-limit:] if limit else []
                    changed = True
                out.append(inst)
            if changed:
                blk.instructions = out


def _build_program(qlen, vlen):
    """Build the SPMD Bass program for the given per-batch lengths."""
    import concourse.bass as bass
    import concourse.mybir as mybir
    import concourse.tile as tile

    bf16 = mybir.dt.bfloat16
    f32 = mybir.dt.float32
    AF = mybir.ActivationFunctionType

    spans = [(_ceil(qlen[b], QSPAN) if vlen[b] > 0 else 0) for b in range(B)]
    kspans = [_ceil(vlen[b], 512) for b in range(B)]
    ktn = [_ceil(vlen[b], P) for b in range(B)]
    active = [b for b in range(B) if spans[b] > 0 and ktn[b] > 0]

    nc = bass.Bass(
        "TRN2",
        target_bir_lowering=False,
        debug=False,
        enable_asserts=False,
        num_devices=NCORES,
    )

    # ---- DRAM tensors ----
    qt_d, kt_d, vt_d, ot_d, den_d = {}, {}, {}, {}, {}
    for b in active:
        qt_d[b] = nc.dram_tensor(f"qt{b}", [DM, spans[b] * QSPAN], bf16,
                                 kind="ExternalInput").ap()
        kt_d[b] = nc.dram_tensor(f"kt{b}", [DM, kspans[b] * 512], bf16,
                                 kind="ExternalInput").ap()
        vt_d[b] = nc.dram_tensor(f"vt{b}", [DM, ktn[b] * P], bf16,
                                 kind="ExternalInput").ap()
        ot_d[b] = nc.dram_tensor(f"ot{b}", [spans[b], P, QSPAN], f32,
                                 kind="ExternalOutput").ap()
        den_d[b] = nc.dram_tensor(f"den{b}", [spans[b], 33, QSPAN], f32,
                                  kind="ExternalOutput").ap()
    wq_d = nc.dram_tensor("wq", [DM, P], bf16, kind="ExternalInput").ap()
    wk_d = nc.dram_tensor("wk", [DM, P], bf16, kind="ExternalInput").ap()
    wv_d = nc.dram_tensor("wv", [DM, P], bf16, kind="ExternalInput").ap()

    with tile.TileContext(nc) as tc:
        with (
            tc.tile_pool(name="wpool", bufs=1) as wpool,
            tc.tile_pool(name="proj", bufs=1) as projpool,
            tc.tile_pool(name="stage", bufs=1) as stage,
            tc.tile_pool(name="sbig", bufs=3) as sbig,
            tc.tile_pool(name="outp", bufs=2) as outp,
            tc.tile_pool(name="pproj", bufs=2, space="PSUM") as pproj,
            tc.tile_pool(name="psc", bufs=2, space="PSUM") as psc,
            tc.tile_pool(name="ppo", bufs=1, space="PSUM") as ppo,
            tc.tile_pool(name="ppd", bufs=1, space="PSUM") as ppd,
        ):
            # Weights for this core's head pair, DM on partitions: [128, 8, 128]
            w_sb = {}
            for name, ap in (("wq", wq_d), ("wk", wk_d), ("wv", wv_d)):
                t = wpool.tile([P, DM // P, P], bf16, tag=f"w_{name}")
                nc.sync.dma_start(t, ap.rearrange("(c p) m -> p c m", p=P))
                w_sb[name] = t
            ones_sb = wpool.tile([P, 1], bf16, tag="ones")
            nc.vector.memset(ones_sb, 1.0)

            for b in active:
                nsp, nksp, nkt = spans[b], kspans[b], ktn[b]
                boundary = nkt - 1 if vlen[b] % P != 0 else -1
                kones_sb = wpool.tile([P, 1], bf16, tag="kones")
                if boundary >= 0:
                    r = vlen[b] - (nkt - 1) * P
                    nc.vector.memset(kones_sb, 0.0)
                    nc.vector.memset(kones_sb[0:r], 1.0)

                qT = projpool.tile([P, nsp * QSPAN], bf16, tag="qT")
                kT = projpool.tile([P, nksp * 512], bf16, tag="kT")
                vn = projpool.tile([P, nkt * P], bf16, tag="vn")

                # Stage full transposed activations with one DMA each.
                qst = stage.tile([P, DM // P, nsp * QSPAN], bf16, tag="qst")
                nc.sync.dma_start(qst, qt_d[b].rearrange("(c p) n -> p c n",
                                                         p=P))
                kst = stage.tile([P, DM // P, nksp * 512], bf16, tag="kst")
                nc.sync.dma_start(kst, kt_d[b].rearrange("(c p) n -> p c n",
                                                         p=P))
                vst = stage.tile([P, DM // P, nkt * P], bf16, tag="vst")
                nc.sync.dma_start(vst, vt_d[b].rearrange("(c p) n -> p c n",
                                                         p=P))

                # q projection: qT[:, span] = (WQ_hp^T @ Q^T)[128, 512]
                for sp in range(nsp):
                    ps = pproj.tile([P, 512], f32, tag="proj")
                    for ch in range(DM // P):
                        nc.tensor.matmul(ps[:, :QSPAN], lhsT=w_sb["wq"][:, ch],
                                         rhs=qst[:, ch,
                                                 sp * QSPAN:(sp + 1) * QSPAN],
                                         start=(ch == 0),
                                         stop=(ch == DM // P - 1))
                    nc.vector.tensor_copy(qT[:, sp * QSPAN:(sp + 1) * QSPAN],
                                          ps[:, :QSPAN])
                # k projection
                for sp in range(nksp):
                    ps = pproj.tile([P, 512], f32, tag="proj")
                    for ch in range(DM // P):
                        nc.tensor.matmul(ps, lhsT=w_sb["wk"][:, ch],
                                         rhs=kst[:, ch,
                                                 sp * 512:(sp + 1) * 512],
                                         start=(ch == 0),
                                         stop=(ch == DM // P - 1))
                    nc.vector.tensor_copy(kT[:, sp * 512:(sp + 1) * 512], ps)
                # v projection: v natural [128 kpos, 128 hp-dims] per ktile
                for kt in range(nkt):
                    ps = pproj.tile([P, 512], f32, tag="proj")
                    for ch in range(DM // P):
                        nc.tensor.matmul(ps[:, :P],
                                         lhsT=vst[:, ch, kt * P:(kt + 1) * P],
                                         rhs=w_sb["wv"][:, ch],
                                         start=(ch == 0),
                                         stop=(ch == DM // P - 1))
                    nc.vector.tensor_copy(vn[:, kt * P:(kt + 1) * P],
                                          ps[:, :P])

                # attention
                for sp in range(nsp):
                    qs = slice(sp * QSPAN, (sp + 1) * QSPAN)
                    po = ppo.tile([P, QSPAN], f32, tag="po")
                    pd = ppd.tile([33, QSPAN], f32, tag="pd")
                    for kt in range(nkt):
                        ks = slice(kt * P, (kt + 1) * P)
                        psc_t = psc.tile([P, 2 * QSPAN], f32, tag="sc")
                        # scoresT = kT_slice^T @ qT : [128 kpos, 512 q], 2 heads
                        nc.tensor.matmul(psc_t[:, 0:QSPAN],
                                         lhsT=kT[0:64, ks], rhs=qT[0:64, qs],
                                         start=True, stop=True,
                                         tile_position=(0, 0))
                        nc.tensor.matmul(psc_t[:, QSPAN:2 * QSPAN],
                                         lhsT=kT[64:P, ks], rhs=qT[64:P, qs],
                                         start=True, stop=True,
                                         tile_position=(64, 0))
                        ex = sbig.tile([P, 2 * QSPAN], bf16, tag="exp")
                        nc.scalar.activation(ex, psc_t, AF.Exp)
                        first, last = (kt == 0), (kt == nkt - 1)
                        ov = kones_sb if kt == boundary else ones_sb
                        nc.tensor.matmul(po[0:64, :],
                                         lhsT=vn[:, kt * P:kt * P + 64],
                                         rhs=ex[:, 0:QSPAN],
                                         start=first, stop=last,
                                         tile_position=(0, 0))
                        nc.tensor.matmul(po[64:P, :],
                                         lhsT=vn[:, kt * P + 64:kt * P + P],
                                         rhs=ex[:, QSPAN:2 * QSPAN],
                                         start=first, stop=last,
                                         tile_position=(0, 64))
                        nc.tensor.matmul(pd[0:1, :], lhsT=ov,
                                         rhs=ex[:, 0:QSPAN],
                                         start=first, stop=last,
                                         tile_position=(0, 0))
                        nc.tensor.matmul(pd[32:33, :], lhsT=ov,
                                         rhs=ex[:, QSPAN:2 * QSPAN],
                                         start=first, stop=last,
                                         tile_position=(0, 32))
                    osb = outp.tile([P, QSPAN], f32, tag="osb")
                    nc.vector.tensor_copy(osb, po)
                    nc.sync.dma_start(ot_d[b][sp], osb)
                    dsb = outp.tile([33, QSPAN], f32, tag="dsb")
                    nc.vector.tensor_copy(dsb[0:1], pd[0:1])
                    nc.vector.tensor_copy(dsb[32:33], pd[32:33])
                    nc.sync.dma_start(den_d[b][sp], dsb)

    _split_excess_waits(nc, mybir)
    return nc, spans, kspans, ktn, active


def kernel(Q_seq, K_seq, V_seq, Q_len, V_len, WQ, WK, WV):
    global LAST_EXEC_NS, LAST_RESULTS, LAST_NC, LAST_IN_MAPS
    import concourse.bass_utils as bass_utils

    Q_seq = np.ascontiguousarray(np.asarray(Q_seq, dtype=np.float32))
    K_seq = np.ascontiguousarray(np.asarray(K_seq, dtype=np.float32))
    V_seq = np.ascontiguousarray(np.asarray(V_seq, dtype=np.float32))
    WQ = np.asarray(WQ, dtype=np.float32)
    WK = np.asarray(WK, dtype=np.float32)
    WV = np.asarray(WV, dtype=np.float32)
    qlen = [int(x) for x in np.asarray(Q_len).ravel()]
    vlen = [int(x) for x in np.asarray(V_len).ravel()]

    bf = ml_dtypes.bfloat16
    out = np.zeros((B, S, H * D), dtype=np.float32)

    # Degenerate batches (V_len==0): reference softmax of an all-masked row
    # is uniform over all S keys -> O row = mean of v rows.
    host_b = [b for b in range(B)
              if vlen[b] == 0 and qlen[b] > 0]
    for b in host_b:
        v = V_seq[b] @ WV
        out[b, :qlen[b], :] = v.mean(axis=0, keepdims=True)

    key = (tuple(qlen), tuple(vlen))
    if key not in _PROGRAM_CACHE:
        _PROGRAM_CACHE[key] = _build_program(qlen, vlen)
    nc, spans, kspans, ktn, active = _PROGRAM_CACHE[key]

    if active:
        WQs = (WQ / math.sqrt(D)).astype(bf)
        WKs = WK.astype(bf)
        WVs = WV.astype(bf)

        # Shared (core-independent) transposed activations, zero-padded.
        shared = {}
        for b in active:
            qt = np.zeros((DM, spans[b] * QSPAN), dtype=bf)
            qt[:, :qlen[b]] = Q_seq[b, :qlen[b], :].T
            kt = np.zeros((DM, kspans[b] * 512), dtype=bf)
            kt[:, :vlen[b]] = K_seq[b, :vlen[b], :].T
            vt = np.zeros((DM, ktn[b] * P), dtype=bf)
            vt[:, :vlen[b]] = V_seq[b, :vlen[b], :].T
            shared[f"qt{b}"] = qt
            shared[f"kt{b}"] = kt
            shared[f"vt{b}"] = vt

        in_maps = []
        for c in range(NCORES):
            m = dict(shared)
            sl = slice(c * P, (c + 1) * P)
            m["wq"] = np.ascontiguousarray(WQs[:, sl])
            m["wk"] = np.ascontiguousarray(WKs[:, sl])
            m["wv"] = np.ascontiguousarray(WVs[:, sl])
            in_maps.append(m)

        trace = bool(int(os.environ.get("KERNEL_TRACE", "0")))
        res = bass_utils.run_bass_kernel_spmd(
            nc, in_maps, core_ids=list(range(NCORES)), trace=trace)
        LAST_EXEC_NS = res.exec_time_ns
        LAST_RESULTS = res
        LAST_NC = nc
        LAST_IN_MAPS = in_maps

        for c in range(NCORES):
            r = res.results[c]
            for b in active:
                ot = r[f"ot{b}"]    # [nsp, 128, 512] unnormalized O^T pair
                den = r[f"den{b}"]  # [nsp, 33, 512]; rows 0 and 32 are real
                for sp in range(spans[b]):
                    q0 = sp * QSPAN
                    n = min(qlen[b], q0 + QSPAN) - q0
                    if n <= 0:
                        continue
                    for h in (0, 1):
                        head = 2 * c + h
                        num = ot[sp, h * 64:(h + 1) * 64, :n]
                        d = den[sp, 32 * h, :n]
                        out[b, q0:q0 + n, head * 64:(head + 1) * 64] = \
                            (num / d[None, :]).T
    return out

